# revision 8
# baseline (speedup 1.0000x reference)
"""Trainium2 Bass kernel for nn_Attention_org_cross (dual-stream channel attention).

Math (reference): B=4, N=4096, C=1024, H=4, D=256.
  Q/K/V per-head projections of S,SKV,T,TKV; channel attention
  scores[b,h] = Q^T K / 32 (contraction over N); probs = softmax(inorm(scores));
  ctx = probs @ V^T; out = merge(ctx) @ Wout^T.  Four attends:
  a0 S_out=(Q,K,V,Wo)  a1 T2S=(Qd,K,V,Wod)  a2 S2T=(Q,Kd,Vd,Wo)  a3 T_out=(Qd,Kd,Vd,Wod)

Sharding: sequence-parallel over N across 8 cores (NL=512 rows/core/batch).
Projections, partial scores, ctx, out-linear are all row-local in N; the only
collective is an AllReduce of the [4,H,D,D] partial score blocks per batch
(bf16, 2MB/batch) — vs 256MB for a context all-gather.

Host-side prep folds all transposes/permutations into the input layout:
  x inputs     -> [B, C, NL] bf16  (channel-major shard, d on partitions)
  wq/wk/wv     -> [H, D, D]  bf16  W^T per head (d, e)
  wo/wod       -> [H, D, C]  bf16  WoP[h][d, o] = Wout[o, d*H + h]
"""

import numpy as np
import ml_dtypes

import concourse.bass as bass
import concourse.mybir as mybir
import concourse.tile as tile
from concourse import bacc
from concourse.masks import make_identity
from concourse.bass_utils import run_bass_kernel_spmd

B, N, C, H = 4, 4096, 1024, 4
D = C // H            # 256
NCORES = 8
NL = N // NCORES      # 512 rows per core per batch
P = 128
EPS = 1e-5
SCALE = 1.0 / 32.0    # 1/sqrt(KV_SIZE=1024)
A = 4                 # four attends

BF16 = mybir.dt.bfloat16
F32 = mybir.dt.float32

# score blocks per (b): (a, h) pairs
BLOCKS = [(a, h) for a in range(A) for h in range(H)]


def build():
    nc = bacc.Bacc("TRN2", target_bir_lowering=False, debug=False,
                   num_devices=NCORES)

    # ---- I/O ----
    xs_in = {}
    for nm in ("xS", "xSKV", "xT", "xTKV"):
        xs_in[nm] = nc.dram_tensor(nm, [B, C, NL], BF16, kind="ExternalInput")
    wq = nc.dram_tensor("wq", [H, D, D], BF16, kind="ExternalInput")
    wk = nc.dram_tensor("wk", [H, D, D], BF16, kind="ExternalInput")
    wv = nc.dram_tensor("wv", [H, D, D], BF16, kind="ExternalInput")
    wo = nc.dram_tensor("wo", [H, D, C], BF16, kind="ExternalInput")
    wod = nc.dram_tensor("wod", [H, D, C], BF16, kind="ExternalInput")
    out_d = nc.dram_tensor("out", [A, B, NL, C], F32, kind="ExternalOutput")

    with tile.TileContext(nc) as tc:
        with (
            tc.tile_pool(name="wpool", bufs=1) as wpool,
            tc.tile_pool(name="vpool", bufs=1) as vpool,
            tc.tile_pool(name="xpool", bufs=1) as xpool,
            tc.tile_pool(name="qkpool", bufs=1) as qkpool,
            tc.tile_pool(name="small", bufs=2) as small,
            tc.tile_pool(name="scpool", bufs=2) as scpool,
            tc.tile_pool(name="ctxpool", bufs=2) as ctxpool,
            tc.tile_pool(name="opool", bufs=2) as opool,
            tc.tile_pool(name="psum_small", bufs=2, space="PSUM") as psum_small,
            tc.tile_pool(name="psum_tiny", bufs=1, space="PSUM") as psum_tiny,
            tc.tile_pool(name="psum_nl", bufs=2, space="PSUM") as psum_nl,
            tc.tile_pool(name="psum_out", bufs=1, space="PSUM") as psum_out,
            tc.tile_pool(name="dram", bufs=1, space="DRAM") as dram,
        ):
            # ---- weights to SBUF ----
            wq_sb = wpool.tile([P, H, 2, D], BF16)
            wk_sb = wpool.tile([P, H, 2, D], BF16)
            wv_sb = wpool.tile([P, H, 2, D], BF16)
            for w_sb, w_d in ((wq_sb, wq), (wk_sb, wk), (wv_sb, wv)):
                nc.sync.dma_start(w_sb[:], w_d.ap().rearrange(
                    "h (dt p) e -> p h dt e", p=P))
            wo_sb = wpool.tile([P, H, 2, C], BF16)
            wod_sb = wpool.tile([P, H, 2, C], BF16)
            nc.sync.dma_start(wo_sb[:], wo.ap().rearrange(
                "h (dt p) o -> p h dt o", p=P))
            nc.sync.dma_start(wod_sb[:], wod.ap().rearrange(
                "h (dt p) o -> p h dt o", p=P))

            ident = wpool.tile([P, P], BF16)
            make_identity(nc, ident[:])
            ones_col = wpool.tile([P, 1], F32)      # [128,1] of 1.0 (lhsT for col-sum)
            nc.vector.memset(ones_col[:], 1.0)
            ones_row = wpool.tile([1, P], F32)      # [1,128] of 1.0 (lhsT for bcast)
            nc.vector.memset(ones_row[:], 1.0)
            eps_sb = wpool.tile([1, 1], F32)
            nc.vector.memset(eps_sb[:], EPS)

            # persistent V/Vt storage: Vt[j(=e), n] per (b,h)
            v_sb = [[None] * H for _ in range(B)]
            vd_sb = [[None] * H for _ in range(B)]

            # score bounce buffers (bf16): [b][a,h,ib,p,j]
            sc_in = dram.tile([B, A, H, 2, P, D], BF16)
            sc_out = [dram.tile([A, H, 2, P, D], BF16, addr_space="Shared",
                                name=f"sc_out_{b}") for b in range(B)]

            # ---------------- phase 1+2: projections, partial scores, AR ----
            def phase2(b):
                q_t = {}
                # S-side then T-side so only two x tiles are live at a time
                for (xa, xb_, qname, kname) in (
                        ("xS", "xSKV", "Q", "K"), ("xT", "xTKV", "Qd", "Kd")):
                    xa_sb = xpool.tile([P, 8, NL], BF16, name=f"x_{xa}", tag="x_a")
                    xb_sb = xpool.tile([P, 8, NL], BF16, name=f"x_{xb_}", tag="x_b")
                    nc.sync.dma_start(xa_sb[:], xs_in[xa][b].rearrange(
                        "(ct p) n -> p ct n", p=P))
                    nc.sync.dma_start(xb_sb[:], xs_in[xb_][b].rearrange(
                        "(ct p) n -> p ct n", p=P))
                    qt = qkpool.tile([P, H, 4, D], BF16, name=f"qk_{qname}")
                    kt = qkpool.tile([P, H, 4, D], BF16, name=f"qk_{kname}")
                    q_t[qname], q_t[kname] = qt, kt
                    vt_all = v_sb if qname == "Q" else vd_sb
                    vpfx = "v" if qname == "Q" else "vd"
                    for h in range(H):
                        vt_all[b][h] = vpool.tile(
                            [P, 2, NL], BF16, name=f"{vpfx}_{b}_{h}",
                            tag=f"{vpfx}_{b % 2}_{h}", bufs=1)
                        # Q/K: out[n(128), e(256)] = x^T.T @ W^T
                        for (dst, xsb, wsb) in ((qt, xa_sb, wq_sb),
                                                (kt, xb_sb, wk_sb)):
                            for nt in range(4):
                                ps = psum_small.tile([P, D], F32, name="ps_proj", tag="ps_small")
                                for dt in range(2):
                                    nc.tensor.matmul(
                                        ps[:],
                                        xsb[:, 2 * h + dt, nt * P:(nt + 1) * P],
                                        wsb[:, h, dt, :],
                                        start=(dt == 0), stop=(dt == 1))
                                nc.vector.tensor_copy(dst[:, h, nt, :], ps[:])
                        # V: out[e(2x128), n(512)] = W^T.T @ x^T
                        for et in range(2):
                            ps = psum_nl.tile([P, NL], F32, name="ps_v", tag="ps_nl")
                            for dt in range(2):
                                nc.tensor.matmul(
                                    ps[:],
                                    wv_sb[:, h, dt, et * P:(et + 1) * P],
                                    xb_sb[:, 2 * h + dt, :],
                                    start=(dt == 0), stop=(dt == 1))
                            nc.vector.tensor_copy(vt_all[b][h][:, et, :], ps[:])

                # partial scores for the 4 attends
                for h in range(H):
                    for a, (qn, kn) in enumerate(
                            (("Q", "K"), ("Qd", "K"), ("Q", "Kd"), ("Qd", "Kd"))):
                        qt, kt = q_t[qn], q_t[kn]
                        for ib in range(2):
                            ps = psum_small.tile([P, D], F32, name="ps_sc", tag="ps_small")
                            for nt in range(4):
                                nc.tensor.matmul(
                                    ps[:],
                                    qt[:, h, nt, ib * P:(ib + 1) * P],
                                    kt[:, h, nt, :],
                                    start=(nt == 0), stop=(nt == 3))
                            stg = small.tile([P, D], BF16, name="sc_stage")
                            nc.vector.tensor_copy(stg[:], ps[:])
                            nc.sync.dma_start(sc_in[b, a, h, ib], stg[:])

                nc.gpsimd.collective_compute(
                    "AllReduce", mybir.AluOpType.add,
                    replica_groups=[list(range(NCORES))],
                    ins=[sc_in[b].opt()], outs=[sc_out[b][:].opt()])

            # ---------------- phase 3: softmax, ctx, out-linear ----
            def phase3(b):
                for a in range(A):
                    scb = scpool.tile([P, H, 2, D], BF16, name="scb")
                    nc.sync.dma_start(scb[:], sc_out[b][a].rearrange(
                        "h ib p j -> p h ib j"))
                    # per-partition sums / sumsq for the 4 head-blocks
                    ssums = small.tile([P, 2, H], F32, name="ssums")
                    sqscr = small.tile([P, 2 * D], BF16, name="sqscr")
                    for h in range(H):
                        flat = scb[:, h].rearrange("p i j -> p (i j)")
                        nc.vector.reduce_sum(ssums[:, 0, h:h + 1], flat,
                                             axis=mybir.AxisListType.X)
                        nc.scalar.activation(
                            sqscr[:], flat,
                            mybir.ActivationFunctionType.Square,
                            accum_out=ssums[:, 1, h:h + 1])
                    stat_ps = psum_tiny.tile([1, 2 * H], F32, name="stat_ps")
                    nc.tensor.matmul(stat_ps[:],
                                     ones_col[:],
                                     ssums[:].rearrange("p a b -> p (a b)"))
                    # scalar math, vectorized over the 4 heads ([1,H] slices)
                    stat = small.tile([1, 2, H], F32, name="stat")
                    nc.vector.tensor_copy(stat[:], stat_ps[:].rearrange(
                        "p (a b) -> p a b", a=2))
                    bcv = small.tile([1, 2, H], F32, name="bcv")
                    mean = small.tile([1, H], F32, name="mean")
                    var = small.tile([1, H], F32, name="var")
                    # mean = S * SCALE/65536 ; E2 = S2 * SCALE^2/65536
                    nc.scalar.mul(mean[:], stat[:, 0, :], SCALE / (D * D))
                    nc.scalar.mul(var[:], stat[:, 1, :], SCALE * SCALE / (D * D))
                    # var = E2 - mean^2
                    msq = small.tile([1, H], F32, name="msq")
                    nc.vector.tensor_mul(msq[:], mean[:], mean[:])
                    nc.vector.tensor_sub(var[:], var[:], msq[:])
                    # rstd = 1/sqrt(var+eps)
                    nc.scalar.activation(var[:], var[:],
                                         mybir.ActivationFunctionType.Sqrt,
                                         bias=eps_sb[:])
                    nc.vector.reciprocal(var[:], var[:])
                    # bcv[0] = rstd*SCALE (exp scale), bcv[1] = -mean*rstd (bias)
                    nc.scalar.mul(bcv[:, 0, :], var[:], SCALE)
                    nc.vector.tensor_mul(bcv[:, 1, :], mean[:], var[:])
                    nc.scalar.mul(bcv[:, 1, :], bcv[:, 1, :], -1.0)
                    bc_ps = psum_tiny.tile([P, 2 * H], F32, name="bc_ps")
                    nc.tensor.matmul(bc_ps[:], ones_row[:],
                                     bcv[:].rearrange("p a b -> p (a b)"))
                    bc = small.tile([P, 2, H], F32, name="bc")
                    nc.vector.tensor_copy(bc[:], bc_ps[:].rearrange(
                        "p (a b) -> p a b", a=2))

                    ctx_h = []
                    for h in range(H):
                        probs = small.tile([P, 2, D], BF16, name="probs")
                        rowsum = small.tile([P, 2], F32, name="rowsum")
                        for ib in range(2):
                            nc.scalar.activation(
                                probs[:, ib, :], scb[:, h, ib, :],
                                mybir.ActivationFunctionType.Exp,
                                bias=bc[:, 1, h:h + 1], scale=bc[:, 0, h:h + 1],
                                accum_out=rowsum[:, ib:ib + 1])
                        nc.vector.reciprocal(rowsum[:], rowsum[:])
                        for ib in range(2):
                            nc.vector.tensor_scalar_mul(
                                probs[:, ib, :], probs[:, ib, :],
                                rowsum[:, ib:ib + 1])
                        # transpose probs -> pT[j, i]
                        pt_ps = psum_small.tile([P, 2, D], BF16, name="pt_ps", tag="ps_small")
                        for it in range(2):
                            for jt in range(2):
                                nc.tensor.transpose(
                                    pt_ps[:, jt, it * P:(it + 1) * P],
                                    probs[:, it, jt * P:(jt + 1) * P],
                                    ident[:])
                        pt = small.tile([P, 2, D], BF16, name="pt")
                        nc.vector.tensor_copy(pt[:], pt_ps[:])
                        # ctx[i(2x128), n(512)] = probs @ V^T
                        vt = (v_sb if a < 2 else vd_sb)[b][h]
                        ctx = ctxpool.tile([P, 2, NL], BF16, name=f"ctx_{h}")
                        ctx_h.append(ctx)
                        for et in range(2):
                            ps = psum_nl.tile([P, NL], F32, name="ps_ctx", tag="ps_nl")
                            for jt in range(2):
                                nc.tensor.matmul(
                                    ps[:],
                                    pt[:, jt, et * P:(et + 1) * P],
                                    vt[:, jt, :],
                                    start=(jt == 0), stop=(jt == 1))
                            nc.vector.tensor_copy(ctx[:, et, :], ps[:])

                    # out-linear: out[nl(128), o(1024)] summed over (h, et)
                    wsb = wo_sb if a % 2 == 0 else wod_sb
                    od = out_d[a, b].rearrange("(nlb p) o -> nlb p o", p=P)
                    for nlb in range(4):
                        ops = psum_out.tile([P, C], F32, name="ps_out")
                        for ob in range(2):
                            k = 0
                            for h in range(H):
                                for et in range(2):
                                    nc.tensor.matmul(
                                        ops[:, ob * NL:(ob + 1) * NL],
                                        ctx_h[h][:, et, nlb * P:(nlb + 1) * P],
                                        wsb[:, h, et, ob * NL:(ob + 1) * NL],
                                        start=(k == 0), stop=(k == 7))
                                    k += 1
                        osb = opool.tile([P, C], F32, name="osb")
                        nc.vector.tensor_copy(osb[:], ops[:])
                        nc.sync.dma_start(od[nlb], osb[:])

            # pipelined emission: phase3(b) between phase2(b+1) and phase2(b+2)
            phase2(0)
            phase2(1)
            phase3(0)
            phase2(2)
            phase3(1)
            phase2(3)
            phase3(2)
            phase3(3)

    nc.compile()
    return nc


_NC_CACHE = None


def _get_nc():
    global _NC_CACHE
    if _NC_CACHE is None:
        _NC_CACHE = build()
    return _NC_CACHE


def _prep_inputs(S, SKV, T, TKV, Wq, Wk, Wv, Wout, Woutd):
    bf = ml_dtypes.bfloat16
    wq_t = np.ascontiguousarray(np.transpose(Wq, (0, 2, 1))).astype(bf)
    wk_t = np.ascontiguousarray(np.transpose(Wk, (0, 2, 1))).astype(bf)
    wv_t = np.ascontiguousarray(np.transpose(Wv, (0, 2, 1))).astype(bf)
    wo_p = np.stack([Wout[:, h::H].T for h in range(H)]).astype(bf)   # [H,D,C]
    wod_p = np.stack([Woutd[:, h::H].T for h in range(H)]).astype(bf)
    in_maps = []
    for k in range(NCORES):
        sl = slice(k * NL, (k + 1) * NL)
        m = {"wq": wq_t, "wk": wk_t, "wv": wv_t, "wo": wo_p, "wod": wod_p}
        for nm, x in (("xS", S), ("xSKV", SKV), ("xT", T), ("xTKV", TKV)):
            m[nm] = np.ascontiguousarray(
                np.transpose(x[:, sl, :], (0, 2, 1))).astype(bf)
        in_maps.append(m)
    return in_maps


def _assemble(results):
    outs = [np.empty((B, N, C), np.float32) for _ in range(A)]
    for k in range(NCORES):
        o = results[k]["out"]  # [A, B, NL, C]
        for a in range(A):
            outs[a][:, k * NL:(k + 1) * NL, :] = o[a]
    return tuple(outs)


def run(trace=False, **inputs):
    nc = _get_nc()
    in_maps = _prep_inputs(**{k: np.asarray(v, np.float32)
                              for k, v in inputs.items()})
    res = run_bass_kernel_spmd(nc, in_maps, core_ids=list(range(NCORES)),
                               trace=trace)
    return _assemble(res.results), res


def kernel(**inputs):
    out, _ = run(trace=False, **inputs)
    return out
